# revision 17
# baseline (speedup 1.0000x reference)
"""Trainium2 Bass kernel for quantized ConvBlock (fake-quant -> conv3x3 -> BN -> relu6 fake-quant).

Strategy
--------
Data-parallel over batch: 32 images -> 4 per NeuronCore x 8 cores.

Math: the reference fake-quantizes activations to the 256-level grid
k*(6/255), k in [0,255], and weights to m*(s/127), m in [-127,127],
s = max|w|.  Both integer grids are exactly representable in bf16, so the
conv reduces to an *integer* matmul accumulated in fp32 PSUM — exact —
and runs at full bf16 TensorE rate.  Per (r,s) tap the 3x3 conv is a
128(Cin) x 128(Cout-half) matmul over pixels; 9 taps accumulate in PSUM.

Rounding: ACT has no rint, so round-to-nearest-even is done on DVE with
the fp32 magic-number trick (v + 1.5*2^23) - 1.5*2^23.

Schedule (v2): the shared DMA ring serializes all HBM traffic, so the
issue order is chosen to minimize the conv start:
  x0-rows[0:9] | weights (6 col-chunks, absmax partials pipelined on DVE
  behind each chunk) | BN consts | x0-rows[9:33] | x0-rows[33:56] | x1..
Weights are PE-transposed in fp32 *before* quantization (PE is idle in
the prologue and this warms the p-state ramp); per-(half,rs) 128x128
quant pairs on DVE then feed the first conv chunk tap-by-tap.  Half-1
transposes/copies/quant are interleaved one-per-chunk into the image-0
half-0 conv sweep.  PSUM->SBUF transpose copies run on ACT; the BN chain
runs on Pool.  Output stores are batched per (img, half, band) on the
Pool SWDGE ring except the staggered final group, which uses SP HWDGE.
"""

import numpy as np

import concourse.bass as bass
import concourse.mybir as mybir
import concourse.tile as tile
from concourse import bacc, bass_isa
from concourse.bass_utils import run_bass_kernel_spmd

# ---- problem constants (hardcoded per contract) ----
N, C, H, W = 32, 128, 56, 56
O = 256
NCORES = 8
NIMG = N // NCORES  # images per core
HP, WP = H + 2, W + 2  # zero-padded input plane
ROWS_PER_CHUNK = 8
NCHUNK = H // ROWS_PER_CHUNK  # 7
FREE = ROWS_PER_CHUNK * W  # 448 <= 512 (one PSUM bank)

MAGIC = 12582912.0  # 1.5 * 2**23 : fp32 RNE round-to-int trick
QA = 42.5  # 255/6
STEP = float(np.float32(6.0 / 255.0))
BN_EPS = 1e-5

f32 = mybir.dt.float32
bf16 = mybir.dt.bfloat16
ALU = mybir.AluOpType
ACTF = mybir.ActivationFunctionType


def _block_rows(b):
    """Quant block b covers unpadded rows: b==0 -> 0..8 (9 rows), b>=1 ->
    8b+1..8b+8 (8 rows).  Conv chunk ch then depends on blocks ch-1, ch
    only (chunk 0 on block 0 only)."""
    if b == 0:
        return 0, 9
    return 8 * b + 1, 8 if b < 6 else 7


def _build_body(tc):
    nc = tc.nc
    xs = nc.dram_tensor("xs", [NIMG, C, H, W], f32, kind="ExternalInput")
    wt = nc.dram_tensor("wt", [O, C, 3, 3], f32, kind="ExternalInput")
    gm = nc.dram_tensor("gm", [O], f32, kind="ExternalInput")
    bt = nc.dram_tensor("bt", [O], f32, kind="ExternalInput")
    mn = nc.dram_tensor("mn", [O], f32, kind="ExternalInput")
    vr = nc.dram_tensor("vr", [O], f32, kind="ExternalInput")
    out = nc.dram_tensor("out", [NIMG, O, H, W], f32, kind="ExternalOutput")

    from contextlib import ExitStack

    with ExitStack() as ctx:
        const = ctx.enter_context(tc.tile_pool(name="const", bufs=1))
        wpool = ctx.enter_context(tc.tile_pool(name="wpool", bufs=1))
        xraw = ctx.enter_context(tc.tile_pool(name="xraw", bufs=2))
        xqp = ctx.enter_context(tc.tile_pool(name="xqp", bufs=1))
        tq = ctx.enter_context(tc.tile_pool(name="tq", bufs=3))
        psum = ctx.enter_context(tc.tile_pool(name="psum", bufs=8, space="PSUM"))
        post = ctx.enter_context(tc.tile_pool(name="post", bufs=4))
        outb = ctx.enter_context(tc.tile_pool(name="outp", bufs=3))

        from concourse.masks import make_identity

        # ACT: dummy Sqrt at t=0 so the (single) activation-table load
        # overlaps the weight DMAs instead of stalling the copy stream
        dummy = const.tile([128, 1], f32)
        nc.vector.memset(dummy[:], 1.0)
        nc.scalar.activation(dummy[:], dummy[:], ACTF.Sqrt)

        identf = const.tile([128, 128], f32)
        make_identity(nc, identf[:])

        # ================= DMA issue order (single shared ring) =============
        # SP HWDGE queue: x0[0:9] | weights (4 col-chunks) | x0[9:33] |
        # x0[33:56].  BN consts go via Pool SWDGE (no HWDGE slot needed).
        xr = {}
        xs_flat = [xs.ap()[i].rearrange("c h w -> c (h w)") for i in range(NIMG)]
        xr[0] = xraw.tile([C, H * W], f32, name="xr")
        nc.sync.dma_start(xr[0][:, 0 : 9 * W], xs_flat[0][:, 0 : 9 * W])

        wt_nat = wt.ap().rearrange("o i h w -> o (i h w)")
        NWCH = 2  # col-chunks per half
        WCH = (C * 9) // NWCH  # 576
        wnat = []
        for h in range(2):
            wn = wpool.tile([128, C * 9], f32, name=f"wnat{h}")
            for cc in range(NWCH):
                nc.sync.dma_start(
                    wn[:, cc * WCH : (cc + 1) * WCH],
                    wt_nat[h * 128 : (h + 1) * 128, cc * WCH : (cc + 1) * WCH],
                )
            wnat.append(wn)

        nc.sync.dma_start(xr[0][:, 9 * W : 33 * W], xs_flat[0][:, 9 * W : 33 * W])
        nc.sync.dma_start(xr[0][:, 33 * W :], xs_flat[0][:, 33 * W :])

        # BN constants on the Pool SWDGE ring: channel (h*128+p) ->
        # partition p, column h.  vrt goes first: the sqrt/rsqrt seed chain
        # must complete before the DVE quant stream starts, or the Tile
        # scheduler interleaves the seed into the critical DVE window.
        gmt = const.tile([128, 2], f32)
        btt = const.tile([128, 2], f32)
        mnt = const.tile([128, 2], f32)
        vrt = const.tile([128, 2], f32)
        nc.gpsimd.dma_start(vrt[:], vr.ap().rearrange("(h p) -> p h", p=128))
        veps = const.tile([128, 2], f32)
        nc.gpsimd.tensor_scalar(veps[:], vrt[:], BN_EPS, None, op0=ALU.add)
        sv = const.tile([128, 2], f32)
        nc.scalar.activation(sv[:], veps[:], ACTF.Sqrt)
        r_scr = const.tile([128, 2], f32)
        r_cur = const.tile([128, 2], f32)
        nc.vector.reciprocal_approx_accurate(r_cur[:], sv[:], r_scr[:])
        nc.gpsimd.dma_start(gmt[:], gm.ap().rearrange("(h p) -> p h", p=128))
        nc.gpsimd.dma_start(btt[:], bt.ap().rearrange("(h p) -> p h", p=128))
        nc.gpsimd.dma_start(mnt[:], mn.ap().rearrange("(h p) -> p h", p=128))

        # ============ PE: fp32 transposes of half-0 weights ================
        # [o, i] -> [i, o] per rs, before quantization; runs during the
        # weight DMA tail and warms the tensor-engine p-state.  Half-1
        # transposes are interleaved into the image-0 half-0 conv sweep.
        wnat_r = [wnat[h][:].rearrange("o (i r) -> o r i", r=9) for h in range(2)]
        trp = {}

        def transpose_rs(h, rs):
            pst = psum.tile([128, 128], f32, name="pst", bufs=4)
            nc.tensor.transpose(pst[:], wnat_r[h][:, rs, :], identf[:])
            trp[(h, rs)] = pst

        wnatT = [wpool.tile([C, 9, 128], f32, name=f"wnatT{h}") for h in range(2)]

        def copy_rs(h, rs):
            nc.scalar.copy(wnatT[h][:, rs, :], trp.pop((h, rs))[:])

        for rs in range(9):
            transpose_rs(0, rs)
        # p-state warmers: PE-only ops with no data deps, queued after the
        # half-0 transposes so the ramp stays hot until the first conv tap
        for _ in range(3):
            warm = psum.tile([128, 128], f32, name="pst", bufs=4)
            nc.tensor.transpose(warm[:], identf[:], identf[:])

        # ACT: PSUM->SBUF copies of half 0
        for rs in range(9):
            copy_rs(0, rs)

        # ================= DVE: quant block 0 of image 0, then absmax ======
        xq = {}
        xq[0] = xqp.tile([C, HP, WP], bf16, name="xq0")

        def quant_block(im, b):
            r0, nr = _block_rows(b)
            nf = nr * W
            t1 = tq.tile([C, 9 * W], f32, name="t1")
            nc.vector.tensor_scalar(
                t1[:, 0:nf], xr[im][:, r0 * W : r0 * W + nf], QA, 0.0,
                op0=ALU.mult, op1=ALU.max,
            )
            t2 = tq.tile([C, 9 * W], f32, name="t2")
            nc.vector.tensor_scalar(
                t2[:, 0:nf], t1[:, 0:nf], 255.0, MAGIC, op0=ALU.min, op1=ALU.add,
            )
            nc.vector.tensor_scalar(
                xq[im][:, r0 + 1 : r0 + 1 + nr, 1 : W + 1],
                t2[:, 0:nf].rearrange("c (h w) -> c h w", w=W),
                MAGIC, None, op0=ALU.subtract,
            )

        quant_block(0, 0)

        # pad ring of xq (Pool) — emitted early so the Pool queue clears the
        # image-0 pads well before the first conv chunk
        def pad_ring(im):
            nc.gpsimd.memset(xq[im][:, 0, :], 0.0)
            nc.gpsimd.memset(xq[im][:, HP - 1, :], 0.0)
            nc.gpsimd.memset(xq[im][:, 1 : HP - 1, 0], 0.0)
            nc.gpsimd.memset(xq[im][:, 1 : HP - 1, WP - 1], 0.0)

        pad_ring(0)

        # absmax partials pipelined behind the 4 weight DMA col-chunks
        parts = const.tile([128, 2 * NWCH], f32)
        for h in range(2):
            for cc in range(NWCH):
                nc.vector.tensor_reduce(
                    parts[:, h * NWCH + cc : h * NWCH + cc + 1],
                    wnat[h][:, cc * WCH : (cc + 1) * WCH],
                    axis=mybir.AxisListType.X, op=ALU.max,
                    apply_absolute_value=True,
                )
        wabs = const.tile([128, 1], f32)
        nc.vector.tensor_reduce(
            wabs[:], parts[:], axis=mybir.AxisListType.X, op=ALU.max,
        )
        smax = const.tile([C, 1], f32)
        nc.gpsimd.partition_all_reduce(
            smax[:], wabs[:], channels=C, reduce_op=bass_isa.ReduceOp.absmax
        )
        # 1/s via approx reciprocal + extra Newton pass
        from concourse.dve_ops import RECIPROCAL_APPROX_NR

        rscr = const.tile([C, 1], f32)
        srcp = const.tile([C, 1], f32)
        nc.vector.reciprocal_approx_accurate(srcp[:], smax[:], rscr[:])
        srcp2 = const.tile([C, 1], f32)
        nc.vector._custom_dve(
            RECIPROCAL_APPROX_NR, out=srcp2[:], in0=smax[:], in1=srcp[:], s0=2.0
        )
        winv = const.tile([C, 1], f32)  # 127/s
        nc.vector.tensor_scalar(winv[:], srcp2[:], 127.0, None, op0=ALU.mult)

        # ======= per-rs weight quant pairs on DVE (feeds conv tap-by-tap) ===
        wq = [wpool.tile([C, 9, 128], bf16, name=f"wq{h}") for h in range(2)]

        def wquant_rs(h, rs, on_act=False):
            wtmp = tq.tile([C, 128], f32, name="wtmp")
            if on_act:
                nc.scalar.activation(
                    wtmp[:], wnatT[h][:, rs, :], ACTF.Copy, bias=MAGIC, scale=winv[:]
                )
                nc.scalar.activation(wq[h][:, rs, :], wtmp[:], ACTF.Copy, bias=-MAGIC)
            else:
                nc.vector.tensor_scalar(
                    wtmp[:], wnatT[h][:, rs, :], winv[:], MAGIC,
                    op0=ALU.mult, op1=ALU.add,
                )
                nc.vector.tensor_scalar(
                    wq[h][:, rs, :], wtmp[:], MAGIC, None, op0=ALU.subtract
                )

        # taps 0-7 fed by DVE, tap 8 by ACT: the first conv chunk consumes
        # taps at ~187ns cadence, slightly faster than DVE alone quantizes
        wquant_rs(0, 8, on_act=True)
        for rs in range(8):
            wquant_rs(0, rs)

        # ====================== BN chain (Pool engine) ======================
        cur = r_cur
        for it in range(2):
            t_sq = const.tile([128, 2], f32, name=f"rs_t{it}")
            nc.gpsimd.tensor_tensor(t_sq[:], cur[:], cur[:], op=ALU.mult)
            t_u = const.tile([128, 2], f32, name=f"rs_u{it}")
            nc.gpsimd.tensor_tensor(t_u[:], veps[:], t_sq[:], op=ALU.mult)
            t_c = const.tile([128, 2], f32, name=f"rs_c{it}")
            nc.gpsimd.tensor_scalar(t_c[:], t_u[:], -0.5, 1.5, op0=ALU.mult, op1=ALU.add)
            t_n = const.tile([128, 2], f32, name=f"rs_n{it}")
            nc.gpsimd.tensor_tensor(t_n[:], cur[:], t_c[:], op=ALU.mult)
            cur = t_n
        bnscale = const.tile([128, 2], f32)
        nc.gpsimd.tensor_tensor(bnscale[:], gmt[:], cur[:], op=ALU.mult)
        # b2 = 42.5 * (beta - mean*bnscale)
        msc = const.tile([128, 2], f32)
        nc.gpsimd.tensor_tensor(msc[:], mnt[:], bnscale[:], op=ALU.mult)
        bmm = const.tile([128, 2], f32)
        nc.gpsimd.tensor_tensor(bmm[:], btt[:], msc[:], op=ALU.subtract)
        b2 = const.tile([128, 2], f32)
        nc.gpsimd.tensor_scalar(b2[:], bmm[:], QA, None, op0=ALU.mult)
        # a2 = bnscale * s/127   (42.5 * 6/255 == 1)
        qs2 = const.tile([128, 1], f32)
        nc.gpsimd.tensor_scalar(qs2[:], smax[:], 1.0 / 127.0, None, op0=ALU.mult)
        a2 = const.tile([128, 2], f32)
        nc.gpsimd.tensor_scalar(a2[:], bnscale[:], qs2[:], None, op0=ALU.mult)

        # remaining image-0 quant blocks (DVE, behind the wq0 feed)
        for b in range(1, NCHUNK):
            quant_block(0, b)

        # ===================== conv + epilogue main loop ====================
        BAND_OF = [0, 0, 0, 0, 1, 1, 1]
        BAND_COLS = [4 * FREE, 3 * FREE]
        BAND_OFF = [0, 4 * FREE]

        def epilogue(im, half, ch, ps, ob, nrw=ROWS_PER_CHUNK, ro=0):
            nf = nrw * W
            band = BAND_OF[ch]
            boff = (ch - (0 if band == 0 else 4)) * FREE + ro * W
            tpost = post.tile([128, FREE], f32, name="tpost")
            nc.scalar.activation(
                tpost[:, 0:nf], ps[:], ACTF.Relu,
                bias=b2[:, half : half + 1], scale=a2[:, half : half + 1],
            )
            u = post.tile([128, FREE], f32, name="u")
            nc.vector.tensor_scalar(
                u[:, 0:nf], tpost[:, 0:nf], 255.0, MAGIC, op0=ALU.min, op1=ALU.add,
            )
            nc.vector.tensor_scalar(
                ob[:, boff : boff + nf], u[:, 0:nf], MAGIC, STEP,
                op0=ALU.subtract, op1=ALU.mult,
            )
            return boff

        def chunk_matmuls(im, half, ch, nrw=ROWS_PER_CHUNK, ro=0):
            ps = psum.tile([128, nrw * W], f32, name="ps", bufs=4)
            rb = ch * ROWS_PER_CHUNK + ro
            for r in range(3):
                for s in range(3):
                    rs = r * 3 + s
                    nc.tensor.matmul(
                        ps[:],
                        wq[half][:, rs, :],
                        xq[im][:, rb + r : rb + r + nrw, s : s + W],
                        start=(rs == 0),
                        stop=(rs == 8),
                    )
            return ps

        def store_band(im, half, band, ob, eng):
            eng.dma_start(
                out.ap()[im, half * 128 : (half + 1) * 128]
                .rearrange("o h w -> o (h w)")[
                    :, BAND_OFF[band] : BAND_OFF[band] + BAND_COLS[band]
                ],
                ob[:, 0 : BAND_COLS[band]],
            )

        def store_piece(im, half, band, ob, boff, nf, eng=None):
            (eng or nc.sync).dma_start(
                out.ap()[im, half * 128 : (half + 1) * 128]
                .rearrange("o h w -> o (h w)")[
                    :, BAND_OFF[band] + boff : BAND_OFF[band] + boff + nf
                ],
                ob[:, boff : boff + nf],
            )

        for im in range(NIMG):
            last_im = im == NIMG - 1
            if im + 1 < NIMG:
                # prefetch next image (2 band DMAs on SP)
                xr[im + 1] = xraw.tile([C, H * W], f32, name="xr")
                nc.sync.dma_start(
                    xr[im + 1][:, 0 : 33 * W], xs_flat[im + 1][:, 0 : 33 * W]
                )
                nc.sync.dma_start(
                    xr[im + 1][:, 33 * W :], xs_flat[im + 1][:, 33 * W :]
                )

            for half in range(2):
                obA = outb.tile([128, 4 * FREE], f32, name="ob")
                obB = outb.tile([128, 4 * FREE], f32, name="ob")
                final_grp = last_im and half == 1
                for ch in range(NCHUNK):
                    band = BAND_OF[ch]
                    ob = obA if band == 0 else obB
                    if final_grp and ch == NCHUNK - 1:
                        # staggered tail: sub-units of 6/1/1 rows on SP HWDGE
                        for ro, nrw in [(0, 6), (6, 1), (7, 1)]:
                            ps = chunk_matmuls(im, half, ch, nrw=nrw, ro=ro)
                            boff = epilogue(im, half, ch, ps, ob, nrw=nrw, ro=ro)
                            store_piece(im, half, 1, ob, boff, nrw * W)
                    else:
                        ps = chunk_matmuls(im, half, ch)
                        if im == 0 and half == 0:
                            # interleave half-1 weight prep, one rs per chunk
                            transpose_rs(1, ch)
                            copy_rs(1, ch)
                            wquant_rs(1, ch)
                            if ch == NCHUNK - 1:
                                for rs in (7, 8):
                                    transpose_rs(1, rs)
                                    copy_rs(1, rs)
                                    wquant_rs(1, rs)
                        boff = epilogue(im, half, ch, ps, ob)
                        if final_grp:
                            # per-chunk stores so no large transfer queues
                            # on the DMA ring near the end
                            store_piece(
                                im, half, band, ob, boff, FREE,
                                eng=nc.gpsimd if ch < 4 else nc.sync,
                            )
                    if not final_grp:
                        if ch == 3:
                            store_band(im, half, 0, obA, nc.gpsimd)
                        elif ch == NCHUNK - 1:
                            store_band(im, half, 1, obB, nc.gpsimd)

                if half == 0 and im + 1 < NIMG:
                    # next image's quant chain between halves
                    xq[im + 1] = xqp.tile([C, HP, WP], bf16, name=f"xq{im + 1}")
                    pad_ring(im + 1)
                    for b in range(NCHUNK):
                        quant_block(im + 1, b)


_CACHED = None


def _get_program():
    global _CACHED
    if _CACHED is None:
        nc = bacc.Bacc(
            "TRN2", target_bir_lowering=False, debug=False, num_devices=NCORES
        )
        with tile.TileContext(nc) as tc:
            _build_body(tc)
        nc.compile()
        _CACHED = nc
    return _CACHED


def run_on_cores(inputs, trace=False, **kw):
    """Run the SPMD kernel; returns (full_output, BassKernelResults)."""
    nc = _get_program()
    x = np.ascontiguousarray(inputs["x"], dtype=np.float32)
    in_maps = []
    for c in range(NCORES):
        in_maps.append(
            {
                "xs": np.ascontiguousarray(x[c * NIMG : (c + 1) * NIMG]),
                "wt": np.ascontiguousarray(inputs["weight"], dtype=np.float32),
                "gm": np.ascontiguousarray(inputs["gamma"], dtype=np.float32),
                "bt": np.ascontiguousarray(inputs["beta"], dtype=np.float32),
                "mn": np.ascontiguousarray(inputs["mean"], dtype=np.float32),
                "vr": np.ascontiguousarray(inputs["var"], dtype=np.float32),
            }
        )
    res = run_bass_kernel_spmd(nc, in_maps, list(range(NCORES)), trace=trace, **kw)
    full = np.concatenate([res.results[c]["out"] for c in range(NCORES)], axis=0)
    return full.astype(np.float32), res


def kernel(**inputs) -> np.ndarray:
    full, _ = run_on_cores(inputs)
    return full


# revision 18
# speedup vs baseline: 1.0049x; 1.0049x over previous
"""Trainium2 Bass kernel for quantized ConvBlock (fake-quant -> conv3x3 -> BN -> relu6 fake-quant).

Strategy
--------
Data-parallel over batch: 32 images -> 4 per NeuronCore x 8 cores.

Math: the reference fake-quantizes activations to the 256-level grid
k*(6/255), k in [0,255], and weights to m*(s/127), m in [-127,127],
s = max|w|.  Both integer grids are exactly representable in bf16, so the
conv reduces to an *integer* matmul accumulated in fp32 PSUM — exact —
and runs at full bf16 TensorE rate.  Per (r,s) tap the 3x3 conv is a
128(Cin) x 128(Cout-half) matmul over pixels; 9 taps accumulate in PSUM.

Rounding: ACT has no rint, so round-to-nearest-even is done on DVE with
the fp32 magic-number trick (v + 1.5*2^23) - 1.5*2^23.

Schedule (v2): the shared DMA ring serializes all HBM traffic, so the
issue order is chosen to minimize the conv start:
  x0-rows[0:9] | weights (6 col-chunks, absmax partials pipelined on DVE
  behind each chunk) | BN consts | x0-rows[9:33] | x0-rows[33:56] | x1..
Weights are PE-transposed in fp32 *before* quantization (PE is idle in
the prologue and this warms the p-state ramp); per-(half,rs) 128x128
quant pairs on DVE then feed the first conv chunk tap-by-tap.  Half-1
transposes/copies/quant are interleaved one-per-chunk into the image-0
half-0 conv sweep.  PSUM->SBUF transpose copies run on ACT; the BN chain
runs on Pool.  Output stores are batched per (img, half, band) on the
Pool SWDGE ring except the staggered final group, which uses SP HWDGE.
"""

import numpy as np

import concourse.bass as bass
import concourse.mybir as mybir
import concourse.tile as tile
from concourse import bacc, bass_isa
from concourse.bass_utils import run_bass_kernel_spmd

# ---- problem constants (hardcoded per contract) ----
N, C, H, W = 32, 128, 56, 56
O = 256
NCORES = 8
NIMG = N // NCORES  # images per core
HP, WP = H + 2, W + 2  # zero-padded input plane
ROWS_PER_CHUNK = 8
NCHUNK = H // ROWS_PER_CHUNK  # 7
FREE = ROWS_PER_CHUNK * W  # 448 <= 512 (one PSUM bank)

MAGIC = 12582912.0  # 1.5 * 2**23 : fp32 RNE round-to-int trick
QA = 42.5  # 255/6
STEP = float(np.float32(6.0 / 255.0))
BN_EPS = 1e-5

f32 = mybir.dt.float32
bf16 = mybir.dt.bfloat16
ALU = mybir.AluOpType
ACTF = mybir.ActivationFunctionType


def _block_rows(b):
    """Quant block b covers unpadded rows: b==0 -> 0..8 (9 rows), b>=1 ->
    8b+1..8b+8 (8 rows).  Conv chunk ch then depends on blocks ch-1, ch
    only (chunk 0 on block 0 only)."""
    if b == 0:
        return 0, 9
    return 8 * b + 1, 8 if b < 6 else 7


def _build_body(tc):
    nc = tc.nc
    xs = nc.dram_tensor("xs", [NIMG, C, H, W], f32, kind="ExternalInput")
    wt = nc.dram_tensor("wt", [O, C, 3, 3], f32, kind="ExternalInput")
    gm = nc.dram_tensor("gm", [O], f32, kind="ExternalInput")
    bt = nc.dram_tensor("bt", [O], f32, kind="ExternalInput")
    mn = nc.dram_tensor("mn", [O], f32, kind="ExternalInput")
    vr = nc.dram_tensor("vr", [O], f32, kind="ExternalInput")
    out = nc.dram_tensor("out", [NIMG, O, H, W], f32, kind="ExternalOutput")

    from contextlib import ExitStack

    with ExitStack() as ctx:
        const = ctx.enter_context(tc.tile_pool(name="const", bufs=1))
        wpool = ctx.enter_context(tc.tile_pool(name="wpool", bufs=1))
        xraw = ctx.enter_context(tc.tile_pool(name="xraw", bufs=2))
        xqp = ctx.enter_context(tc.tile_pool(name="xqp", bufs=1))
        tq = ctx.enter_context(tc.tile_pool(name="tq", bufs=3))
        psum = ctx.enter_context(tc.tile_pool(name="psum", bufs=8, space="PSUM"))
        post = ctx.enter_context(tc.tile_pool(name="post", bufs=4))
        outb = ctx.enter_context(tc.tile_pool(name="outp", bufs=3))

        from concourse.masks import make_identity

        # ACT: dummy Sqrt at t=0 so the (single) activation-table load
        # overlaps the weight DMAs instead of stalling the copy stream
        dummy = const.tile([128, 1], f32)
        nc.vector.memset(dummy[:], 1.0)
        nc.scalar.activation(dummy[:], dummy[:], ACTF.Sqrt)

        identf = const.tile([128, 128], f32)
        make_identity(nc, identf[:])

        # ================= DMA issue order (single shared ring) =============
        # SP HWDGE queue: x0[0:9] | weights (4 col-chunks) | x0[9:33] |
        # x0[33:56].  BN consts go via Pool SWDGE (no HWDGE slot needed).
        xr = {}
        xs_flat = [xs.ap()[i].rearrange("c h w -> c (h w)") for i in range(NIMG)]
        xr[0] = xraw.tile([C, H * W], f32, name="xr")
        nc.sync.dma_start(xr[0][:, 0 : 9 * W], xs_flat[0][:, 0 : 9 * W])

        wt_nat = wt.ap().rearrange("o i h w -> o (i h w)")
        NWCH = 2  # col-chunks per half
        WCH = (C * 9) // NWCH  # 576
        wnat = []
        for h in range(2):
            wn = wpool.tile([128, C * 9], f32, name=f"wnat{h}")
            for cc in range(NWCH):
                nc.sync.dma_start(
                    wn[:, cc * WCH : (cc + 1) * WCH],
                    wt_nat[h * 128 : (h + 1) * 128, cc * WCH : (cc + 1) * WCH],
                )
            wnat.append(wn)

        nc.sync.dma_start(xr[0][:, 9 * W : 33 * W], xs_flat[0][:, 9 * W : 33 * W])
        nc.sync.dma_start(xr[0][:, 33 * W :], xs_flat[0][:, 33 * W :])

        # BN constants on the Pool SWDGE ring: channel (h*128+p) ->
        # partition p, column h.  vrt goes first: the sqrt/rsqrt seed chain
        # must complete before the DVE quant stream starts, or the Tile
        # scheduler interleaves the seed into the critical DVE window.
        gmt = const.tile([128, 2], f32)
        btt = const.tile([128, 2], f32)
        mnt = const.tile([128, 2], f32)
        vrt = const.tile([128, 2], f32)
        nc.gpsimd.dma_start(vrt[:], vr.ap().rearrange("(h p) -> p h", p=128))
        veps = const.tile([128, 2], f32)
        sv = const.tile([128, 2], f32)
        r_scr = const.tile([128, 2], f32)
        r_cur = const.tile([128, 2], f32)
        with tc.high_priority():
            nc.gpsimd.tensor_scalar(veps[:], vrt[:], BN_EPS, None, op0=ALU.add)
            nc.scalar.activation(sv[:], veps[:], ACTF.Sqrt)
            nc.vector.reciprocal_approx_accurate(r_cur[:], sv[:], r_scr[:])
        nc.gpsimd.dma_start(gmt[:], gm.ap().rearrange("(h p) -> p h", p=128))
        nc.gpsimd.dma_start(btt[:], bt.ap().rearrange("(h p) -> p h", p=128))
        nc.gpsimd.dma_start(mnt[:], mn.ap().rearrange("(h p) -> p h", p=128))

        # ============ PE: fp32 transposes of half-0 weights ================
        # [o, i] -> [i, o] per rs, before quantization; runs during the
        # weight DMA tail and warms the tensor-engine p-state.  Half-1
        # transposes are interleaved into the image-0 half-0 conv sweep.
        wnat_r = [wnat[h][:].rearrange("o (i r) -> o r i", r=9) for h in range(2)]
        trp = {}
        TR_PAIRS = [(0, 1), (2, 3), (4, 5), (6, 7), (8,)]

        def transpose_pair(h, pair):
            pst = psum.tile([128, 256], f32, name="pst", bufs=3)
            for j, rs in enumerate(pair):
                nc.tensor.transpose(
                    pst[:, j * 128 : (j + 1) * 128], wnat_r[h][:, rs, :], identf[:]
                )
            trp[(h, pair)] = pst

        wnatT = [wpool.tile([C, 9, 128], f32, name=f"wnatT{h}") for h in range(2)]

        def copy_pair(h, pair):
            n = len(pair)
            nc.scalar.copy(
                wnatT[h][:, pair[0] : pair[0] + n, :],
                trp.pop((h, pair))[:, 0 : n * 128].rearrange(
                    "c (n i) -> c n i", n=n
                ),
            )

        for pair in TR_PAIRS:
            transpose_pair(0, pair)

        # ACT: PSUM->SBUF copies of half 0
        for pair in TR_PAIRS:
            copy_pair(0, pair)

        # ================= DVE: quant block 0 of image 0, then absmax ======
        xq = {}
        xq[0] = xqp.tile([C, HP, WP], bf16, name="xq0")

        def quant_block(im, b):
            r0, nr = _block_rows(b)
            nf = nr * W
            t1 = tq.tile([C, 9 * W], f32, name="t1")
            nc.vector.tensor_scalar(
                t1[:, 0:nf], xr[im][:, r0 * W : r0 * W + nf], QA, 0.0,
                op0=ALU.mult, op1=ALU.max,
            )
            t2 = tq.tile([C, 9 * W], f32, name="t2")
            nc.vector.tensor_scalar(
                t2[:, 0:nf], t1[:, 0:nf], 255.0, MAGIC, op0=ALU.min, op1=ALU.add,
            )
            nc.vector.tensor_scalar(
                xq[im][:, r0 + 1 : r0 + 1 + nr, 1 : W + 1],
                t2[:, 0:nf].rearrange("c (h w) -> c h w", w=W),
                MAGIC, None, op0=ALU.subtract,
            )

        quant_block(0, 0)

        # pad ring of xq (Pool) — emitted early so the Pool queue clears the
        # image-0 pads well before the first conv chunk
        def pad_ring(im):
            nc.gpsimd.memset(xq[im][:, 0, :], 0.0)
            nc.gpsimd.memset(xq[im][:, HP - 1, :], 0.0)
            nc.gpsimd.memset(xq[im][:, 1 : HP - 1, 0], 0.0)
            nc.gpsimd.memset(xq[im][:, 1 : HP - 1, WP - 1], 0.0)

        pad_ring(0)

        # absmax partials pipelined behind the 4 weight DMA col-chunks
        parts = const.tile([128, 2 * NWCH], f32)
        for h in range(2):
            for cc in range(NWCH):
                nc.vector.tensor_reduce(
                    parts[:, h * NWCH + cc : h * NWCH + cc + 1],
                    wnat[h][:, cc * WCH : (cc + 1) * WCH],
                    axis=mybir.AxisListType.X, op=ALU.max,
                    apply_absolute_value=True,
                )
        wabs = const.tile([128, 1], f32)
        nc.vector.tensor_reduce(
            wabs[:], parts[:], axis=mybir.AxisListType.X, op=ALU.max,
        )
        smax = const.tile([C, 1], f32)
        nc.gpsimd.partition_all_reduce(
            smax[:], wabs[:], channels=C, reduce_op=bass_isa.ReduceOp.absmax
        )
        # 1/s via approx reciprocal + extra Newton pass
        from concourse.dve_ops import RECIPROCAL_APPROX_NR

        rscr = const.tile([C, 1], f32)
        srcp = const.tile([C, 1], f32)
        nc.vector.reciprocal_approx_accurate(srcp[:], smax[:], rscr[:])
        srcp2 = const.tile([C, 1], f32)
        nc.vector._custom_dve(
            RECIPROCAL_APPROX_NR, out=srcp2[:], in0=smax[:], in1=srcp[:], s0=2.0
        )
        winv = const.tile([C, 1], f32)  # 127/s
        nc.vector.tensor_scalar(winv[:], srcp2[:], 127.0, None, op0=ALU.mult)

        # ======= per-rs weight quant pairs on DVE (feeds conv tap-by-tap) ===
        wq = [wpool.tile([C, 9, 128], bf16, name=f"wq{h}") for h in range(2)]

        def wquant_pair(h, pair, on_act=False):
            n = len(pair)
            wtmp = tq.tile([C, 2, 128], f32, name="wtmp")
            src = wnatT[h][:, pair[0] : pair[0] + n, :]
            dst = wq[h][:, pair[0] : pair[0] + n, :]
            if on_act:
                nc.scalar.activation(
                    wtmp[:, 0:n, :], src, ACTF.Copy, bias=MAGIC, scale=winv[:]
                )
                nc.scalar.activation(dst, wtmp[:, 0:n, :], ACTF.Copy, bias=-MAGIC)
            else:
                nc.vector.tensor_scalar(
                    wtmp[:, 0:n, :], src, winv[:], MAGIC, op0=ALU.mult, op1=ALU.add,
                )
                nc.vector.tensor_scalar(
                    dst, wtmp[:, 0:n, :], MAGIC, None, op0=ALU.subtract
                )

        # tap-pairs 0-7 fed by DVE, tap 8 by ACT: the first conv chunk
        # consumes taps at ~187ns cadence
        wquant_pair(0, (8,), on_act=True)
        for pair in TR_PAIRS[:4]:
            wquant_pair(0, pair)

        # ====================== BN chain (Pool engine) ======================
        cur = r_cur
        for it in range(2):
            t_sq = const.tile([128, 2], f32, name=f"rs_t{it}")
            nc.gpsimd.tensor_tensor(t_sq[:], cur[:], cur[:], op=ALU.mult)
            t_u = const.tile([128, 2], f32, name=f"rs_u{it}")
            nc.gpsimd.tensor_tensor(t_u[:], veps[:], t_sq[:], op=ALU.mult)
            t_c = const.tile([128, 2], f32, name=f"rs_c{it}")
            nc.gpsimd.tensor_scalar(t_c[:], t_u[:], -0.5, 1.5, op0=ALU.mult, op1=ALU.add)
            t_n = const.tile([128, 2], f32, name=f"rs_n{it}")
            nc.gpsimd.tensor_tensor(t_n[:], cur[:], t_c[:], op=ALU.mult)
            cur = t_n
        bnscale = const.tile([128, 2], f32)
        nc.gpsimd.tensor_tensor(bnscale[:], gmt[:], cur[:], op=ALU.mult)
        # b2 = 42.5 * (beta - mean*bnscale)
        msc = const.tile([128, 2], f32)
        nc.gpsimd.tensor_tensor(msc[:], mnt[:], bnscale[:], op=ALU.mult)
        bmm = const.tile([128, 2], f32)
        nc.gpsimd.tensor_tensor(bmm[:], btt[:], msc[:], op=ALU.subtract)
        b2 = const.tile([128, 2], f32)
        nc.gpsimd.tensor_scalar(b2[:], bmm[:], QA, None, op0=ALU.mult)
        # a2 = bnscale * s/127   (42.5 * 6/255 == 1)
        qs2 = const.tile([128, 1], f32)
        nc.gpsimd.tensor_scalar(qs2[:], smax[:], 1.0 / 127.0, None, op0=ALU.mult)
        a2 = const.tile([128, 2], f32)
        nc.gpsimd.tensor_scalar(a2[:], bnscale[:], qs2[:], None, op0=ALU.mult)

        # remaining image-0 quant blocks (DVE, behind the wq0 feed)
        for b in range(1, NCHUNK):
            quant_block(0, b)

        # ===================== conv + epilogue main loop ====================
        BAND_OF = [0, 0, 0, 0, 1, 1, 1]
        BAND_COLS = [4 * FREE, 3 * FREE]
        BAND_OFF = [0, 4 * FREE]

        def epilogue(im, half, ch, ps, ob, nrw=ROWS_PER_CHUNK, ro=0):
            nf = nrw * W
            band = BAND_OF[ch]
            boff = (ch - (0 if band == 0 else 4)) * FREE + ro * W
            tpost = post.tile([128, FREE], f32, name="tpost")
            nc.scalar.activation(
                tpost[:, 0:nf], ps[:], ACTF.Relu,
                bias=b2[:, half : half + 1], scale=a2[:, half : half + 1],
            )
            u = post.tile([128, FREE], f32, name="u")
            nc.vector.tensor_scalar(
                u[:, 0:nf], tpost[:, 0:nf], 255.0, MAGIC, op0=ALU.min, op1=ALU.add,
            )
            nc.vector.tensor_scalar(
                ob[:, boff : boff + nf], u[:, 0:nf], MAGIC, STEP,
                op0=ALU.subtract, op1=ALU.mult,
            )
            return boff

        def chunk_matmuls(im, half, ch, nrw=ROWS_PER_CHUNK, ro=0):
            ps = psum.tile([128, nrw * W], f32, name="ps", bufs=4)
            rb = ch * ROWS_PER_CHUNK + ro
            for r in range(3):
                for s in range(3):
                    rs = r * 3 + s
                    nc.tensor.matmul(
                        ps[:],
                        wq[half][:, rs, :],
                        xq[im][:, rb + r : rb + r + nrw, s : s + W],
                        start=(rs == 0),
                        stop=(rs == 8),
                    )
            return ps

        def store_band(im, half, band, ob, eng):
            eng.dma_start(
                out.ap()[im, half * 128 : (half + 1) * 128]
                .rearrange("o h w -> o (h w)")[
                    :, BAND_OFF[band] : BAND_OFF[band] + BAND_COLS[band]
                ],
                ob[:, 0 : BAND_COLS[band]],
            )

        def store_piece(im, half, band, ob, boff, nf, eng=None):
            (eng or nc.sync).dma_start(
                out.ap()[im, half * 128 : (half + 1) * 128]
                .rearrange("o h w -> o (h w)")[
                    :, BAND_OFF[band] + boff : BAND_OFF[band] + boff + nf
                ],
                ob[:, boff : boff + nf],
            )

        for im in range(NIMG):
            last_im = im == NIMG - 1
            if im + 1 < NIMG:
                # prefetch next image (2 band DMAs on SP)
                xr[im + 1] = xraw.tile([C, H * W], f32, name="xr")
                nc.sync.dma_start(
                    xr[im + 1][:, 0 : 33 * W], xs_flat[im + 1][:, 0 : 33 * W]
                )
                nc.sync.dma_start(
                    xr[im + 1][:, 33 * W :], xs_flat[im + 1][:, 33 * W :]
                )

            for half in range(2):
                obA = outb.tile([128, 4 * FREE], f32, name="ob")
                obB = outb.tile([128, 4 * FREE], f32, name="ob")
                final_grp = last_im and half == 1
                for ch in range(NCHUNK):
                    band = BAND_OF[ch]
                    ob = obA if band == 0 else obB
                    if final_grp and ch == NCHUNK - 1:
                        # staggered tail: sub-units of 6/1/1 rows on SP HWDGE
                        for ro, nrw in [(0, 6), (6, 1), (7, 1)]:
                            ps = chunk_matmuls(im, half, ch, nrw=nrw, ro=ro)
                            boff = epilogue(im, half, ch, ps, ob, nrw=nrw, ro=ro)
                            store_piece(im, half, 1, ob, boff, nrw * W)
                    else:
                        ps = chunk_matmuls(im, half, ch)
                        if im == 0 and half == 0 and ch < len(TR_PAIRS):
                            # interleave half-1 weight prep, one tap-pair
                            # per chunk
                            transpose_pair(1, TR_PAIRS[ch])
                            copy_pair(1, TR_PAIRS[ch])
                            wquant_pair(1, TR_PAIRS[ch])
                        boff = epilogue(im, half, ch, ps, ob)
                        if final_grp:
                            # per-chunk stores so no large transfer queues
                            # on the DMA ring near the end
                            store_piece(
                                im, half, band, ob, boff, FREE,
                                eng=nc.gpsimd if ch < 4 else nc.sync,
                            )
                    if not final_grp:
                        if ch == 3:
                            store_band(im, half, 0, obA, nc.gpsimd)
                        elif ch == NCHUNK - 1:
                            store_band(im, half, 1, obB, nc.gpsimd)

                if half == 0 and im + 1 < NIMG:
                    # next image's quant chain between halves
                    xq[im + 1] = xqp.tile([C, HP, WP], bf16, name=f"xq{im + 1}")
                    pad_ring(im + 1)
                    for b in range(NCHUNK):
                        quant_block(im + 1, b)


_CACHED = None


def _get_program():
    global _CACHED
    if _CACHED is None:
        nc = bacc.Bacc(
            "TRN2", target_bir_lowering=False, debug=False, num_devices=NCORES
        )
        with tile.TileContext(nc) as tc:
            _build_body(tc)
        nc.compile()
        _CACHED = nc
    return _CACHED


def run_on_cores(inputs, trace=False, **kw):
    """Run the SPMD kernel; returns (full_output, BassKernelResults)."""
    nc = _get_program()
    x = np.ascontiguousarray(inputs["x"], dtype=np.float32)
    in_maps = []
    for c in range(NCORES):
        in_maps.append(
            {
                "xs": np.ascontiguousarray(x[c * NIMG : (c + 1) * NIMG]),
                "wt": np.ascontiguousarray(inputs["weight"], dtype=np.float32),
                "gm": np.ascontiguousarray(inputs["gamma"], dtype=np.float32),
                "bt": np.ascontiguousarray(inputs["beta"], dtype=np.float32),
                "mn": np.ascontiguousarray(inputs["mean"], dtype=np.float32),
                "vr": np.ascontiguousarray(inputs["var"], dtype=np.float32),
            }
        )
    res = run_bass_kernel_spmd(nc, in_maps, list(range(NCORES)), trace=trace, **kw)
    full = np.concatenate([res.results[c]["out"] for c in range(NCORES)], axis=0)
    return full.astype(np.float32), res


def kernel(**inputs) -> np.ndarray:
    full, _ = run_on_cores(inputs)
    return full
